# revision 13
# baseline (speedup 1.0000x reference)
"""Trainium2 Bass kernel: sampled logistic-regression forward.

reference math (per data row i, sample s):
    mean_i = X[i] . w_mu
    var_i  = sum_d X[i,d]^2 * exp(w_log_var[d])
    out[i,s] = sigmoid( sqrt(var_i) * z[s] + mean_i )

Full shapes: X [500000, 64], w_mu [64], w_log_var [64], z [128]
Output: [500000, 128] fp32.

Sharding: data-parallel over 8 NeuronCores, 62500 rows each.

Layout: chunk-local stripe. Chunk c covers shard rows
[c*125*SC, c*125*SC + 125*TC) -- one contiguous DRAM range per DMA --
and within the chunk, partition p holds rows chunk_base + p*TC + t.
Each DMA descriptor is a per-partition contiguous run of TC rows
(12-24 KB) and each DMA instruction's DRAM side is one contiguous
1.5-3 MB range. Streaming DMAs are additionally split into a
120-partition instruction + 5-partition instruction: descriptor counts
divisible by 15 fan out across all 15 DMA engines (125-descriptor
transfers were observed pinned to 5 engines at ~27 GB/s each).

Per-core pipeline, super-chunks of SC=48 tiles x [125 rows, 64]:
  - DMA in X chunk (contiguous, 120+5 split)
  - ACT: X2 = Square(X)            (sigmoid_and_others table set)
  - DVE: A = X * w_mu (materialized rep); reduce A -> mean;
    reduce V -> var
  - GPSIMD: V = X2 * exp(lv) (rep, in place); Newton rsqrt
    (bit-trick seed, 2 iters); std = var * y
  - stats split hi/lo to f32r precision (mantissa AND-mask) into a
    k-major stats block [125, 5, SC] (contiguous writes)
  - per 24-tile block: PE transpose stats (f32r identity, full rate)
    -> [120, 125]; full-rate float32r matmuls vs constant
    block-diagonal Z2 [5*T, T*128] with rows ordered k*T+t:
      arg = mh*1 + ml*1 + sh*zh + sh*zl + sl*zh  ~= mean + std*z (~2^-24)
  - ACT: Sigmoid [125, 1024] PSUM->SBUF (paired matmul banks)
  - DMA out chunk (contiguous, 120+5 split)
"""

from contextlib import ExitStack

import numpy as np

import concourse.bacc as bacc
import concourse.bass as bass
import concourse.tile as tile
from concourse import mybir
from concourse.bass_utils import run_bass_kernel_spmd

N_CORES = 8
D = 64
NS = 128
P = 125          # rows per tile (partition dim)
SC = 48          # tiles per super-chunk (DMA + stats granularity)
BLK_T = 24       # tiles per matmul block (5*24 = 120 = K of the affine matmul)
SIG_T = 4        # tiles per PSUM bank (4*128 = 512 f32)
KR = 5           # K-rows per tile: mh, ml, sh(*zh), sh(*zl), sl(*zh)
PSPLIT = 120     # descriptor-count split: 120 (15 engines) + 5

RSQRT_MAGIC = 0x5F3759DF
F32R_MASK = 0xFFFFF000   # keep 11 explicit mantissa bits (f32r-representable)
F32 = mybir.dt.float32
F32R = mybir.dt.float32r
BF16 = mybir.dt.bfloat16
U32 = mybir.dt.uint32


def _split_dma(nc, out_tile, in_ap, eng=None):
    eng = eng or nc.sync
    eng.dma_start(out=out_tile[0:PSPLIT], in_=in_ap[0:PSPLIT])
    eng.dma_start(out=out_tile[PSPLIT:P], in_=in_ap[PSPLIT:P])


def build_program(rows: int):
    """Build the single-core Bass/Tile program for `rows` rows (SPMD across cores)."""
    assert rows % P == 0
    ntiles = rows // P
    assert ntiles % SIG_T == 0
    RT = ntiles % BLK_T          # rump matmul-block size (0 -> none)

    nc = bacc.Bacc(
        "TRN2",
        target_bir_lowering=False,
        debug=False,
        num_devices=N_CORES,
    )

    x = nc.dram_tensor("x", [rows, D], F32, kind="ExternalInput")
    wmu_d = nc.dram_tensor("wmu", [P, D], F32, kind="ExternalInput")
    elv_d = nc.dram_tensor("elv", [P, D], F32, kind="ExternalInput")
    z2a_d = nc.dram_tensor("z2a", [KR * BLK_T, BLK_T * NS], F32R, kind="ExternalInput")
    ident = nc.dram_tensor("ident", [P, P], F32, kind="ExternalInput")
    out = nc.dram_tensor("out", [rows, NS], F32, kind="ExternalOutput")

    with tile.TileContext(nc) as tc, ExitStack() as ctx:
        singles = ctx.enter_context(tc.tile_pool(name="singles", bufs=1))
        xin = ctx.enter_context(tc.tile_pool(name="xin", bufs=4))
        sqp = ctx.enter_context(tc.tile_pool(name="sqp", bufs=2))
        amp = ctx.enter_context(tc.tile_pool(name="amp", bufs=2))
        statp = ctx.enter_context(tc.tile_pool(name="statp", bufs=4))
        smalls = ctx.enter_context(tc.tile_pool(name="smalls", bufs=5))
        s2p = ctx.enter_context(tc.tile_pool(name="s2p", bufs=4))
        outp = ctx.enter_context(tc.tile_pool(name="outp", bufs=2))
        pst_pool = ctx.enter_context(tc.tile_pool(name="pst", bufs=2, space="PSUM"))
        paff_pool = ctx.enter_context(tc.tile_pool(name="paff", bufs=3, space="PSUM"))

        # one-time loads; weight vectors are materialized as full [P, SC, D]
        # tensors so the big per-chunk muls avoid stride-0 broadcast APs
        wmu_stage = singles.tile([P, 1, D], F32)
        nc.sync.dma_start(out=wmu_stage, in_=wmu_d.rearrange("p (o d) -> p o d", d=D))
        wmu_rep = singles.tile([P, SC, D], F32)
        nc.vector.tensor_copy(wmu_rep, wmu_stage.to_broadcast([P, SC, D]))
        elv_stage = singles.tile([P, 1, D], F32)
        nc.sync.dma_start(out=elv_stage, in_=elv_d.rearrange("p (o d) -> p o d", d=D))
        elv_rep = singles.tile([P, SC, D], F32)
        nc.gpsimd.tensor_copy(elv_rep, elv_stage.to_broadcast([P, SC, D]))
        z2a_sb = singles.tile([KR * BLK_T, BLK_T * NS], F32R)
        nc.sync.dma_start(out=z2a_sb, in_=z2a_d[:, :])
        id_sb = singles.tile([P, P], F32)
        nc.sync.dma_start(out=id_sb, in_=ident[:, :])
        magic_sb = singles.tile([P, SC], U32)
        nc.vector.memset(magic_sb, RSQRT_MAGIC)
        one_sb = singles.tile([P, 1], U32)
        nc.vector.memset(one_sb, 1)
        mask_sb = singles.tile([P, 1], U32)
        nc.vector.memset(mask_sb, F32R_MASK)

        for c0 in range(0, ntiles, SC):
            TC = min(SC, ntiles - c0)
            row0 = c0 * P
            xc = x[row0 : row0 + P * TC, :].rearrange("(p t) d -> p t d", p=P)
            oc = out[row0 : row0 + P * TC, :].rearrange("(p t) s -> p t s", p=P)

            xt = xin.tile([P, SC, D], F32)
            _split_dma(nc, xt[:, :TC, :], xc, eng=nc.scalar)

            # X^2 on ACT (Square lives in the sigmoid table set)
            x2 = sqp.tile([P, SC, D], F32)
            nc.scalar.activation(
                out=x2[:, :TC, :], in_=xt[:, :TC, :],
                func=mybir.ActivationFunctionType.Square,
            )
            # A = X * w_mu on DVE
            at = amp.tile([P, SC, D], F32)
            nc.vector.tensor_mul(at[:, :TC, :], xt[:, :TC, :], wmu_rep[:, :TC, :])
            # V = X^2 * exp(lv) in place on GPSIMD
            nc.gpsimd.tensor_mul(
                x2[:, :TC, :], x2[:, :TC, :], elv_rep[:, :TC, :]
            )

            mean_t = smalls.tile([P, SC], F32)
            nc.vector.tensor_reduce(
                out=mean_t[:, :TC],
                in_=at[:, :TC, :],
                axis=mybir.AxisListType.X,
                op=mybir.AluOpType.add,
            )
            var = smalls.tile([P, SC], F32)
            nc.vector.tensor_reduce(
                out=var[:, :TC],
                in_=x2[:, :TC, :],
                axis=mybir.AxisListType.X,
                op=mybir.AluOpType.add,
            )

            # y = rsqrt(var) on GPSIMD: seed 0x5f3759df - (bits >> 1), 2 NR iters
            vb = var[:, :TC].bitcast(U32)
            yb = smalls.tile([P, SC], U32)
            nc.vector.tensor_scalar(
                yb[:, :TC], vb, one_sb[:, 0:1], None,
                op0=mybir.AluOpType.logical_shift_right,
            )
            nc.vector.scalar_tensor_tensor(
                out=yb[:, :TC],
                in0=magic_sb[:, :TC],
                scalar=0,
                in1=yb[:, :TC],
                op0=mybir.AluOpType.bypass,
                op1=mybir.AluOpType.subtract,
            )
            y = yb.bitcast(F32)
            t2 = smalls.tile([P, SC], F32)
            for _ in range(2):
                # y <- y*(1.5 - 0.5*var*y^2), via u=y*y; h=(u*-0.5)*var;
                # y=(h+1.5)*y
                nc.gpsimd.tensor_mul(t2[:, :TC], y[:, :TC], y[:, :TC])
                nc.vector.scalar_tensor_tensor(
                    out=t2[:, :TC], in0=t2[:, :TC], scalar=-0.5, in1=var[:, :TC],
                    op0=mybir.AluOpType.mult, op1=mybir.AluOpType.mult,
                )
                nc.vector.scalar_tensor_tensor(
                    out=y[:, :TC], in0=t2[:, :TC], scalar=1.5, in1=y[:, :TC],
                    op0=mybir.AluOpType.add, op1=mybir.AluOpType.mult,
                )
            std_t = smalls.tile([P, SC], F32)
            nc.gpsimd.tensor_mul(std_t[:, :TC], var[:, :TC], y[:, :TC])

            # split mean/std into f32r-representable hi/lo rows:
            # statblk rows per tile: [mh, ml, sh, sh, sl] (t-major, k fastest)
            statblk = statp.tile([P, SC, KR], F32)
            sb_u = statblk.bitcast(U32)
            rem = smalls.tile([P, SC], F32)
            rem2 = smalls.tile([P, SC], F32)
            nc.vector.tensor_scalar(
                sb_u[:, :TC, 0], mean_t[:, :TC].bitcast(U32), mask_sb[:, 0:1], None,
                op0=mybir.AluOpType.bitwise_and,
            )
            nc.vector.tensor_sub(rem[:, :TC], mean_t[:, :TC], statblk[:, :TC, 0])
            nc.vector.tensor_scalar(
                sb_u[:, :TC, 1], rem[:, :TC].bitcast(U32), mask_sb[:, 0:1], None,
                op0=mybir.AluOpType.bitwise_and,
            )
            nc.vector.tensor_scalar(
                sb_u[:, :TC, 2], std_t[:, :TC].bitcast(U32), mask_sb[:, 0:1], None,
                op0=mybir.AluOpType.bitwise_and,
            )
            # same-engine as the sh mask write: the strided k-slice
            # cross-engine dependency is not reliably enforced (observed
            # stale row-3 reads when these ran on gpsimd)
            nc.vector.tensor_copy(sb_u[:, :TC, 3], sb_u[:, :TC, 2])
            nc.vector.tensor_sub(rem2[:, :TC], std_t[:, :TC], statblk[:, :TC, 2])
            nc.vector.tensor_scalar(
                sb_u[:, :TC, 4], rem2[:, :TC].bitcast(U32), mask_sb[:, 0:1], None,
                op0=mybir.AluOpType.bitwise_and,
            )

            outb = outp.tile([P, SC, NS], F32)
            for b0 in range(0, TC, BLK_T):
                T = min(BLK_T, TC - b0)
                tb = KR * T
                z2_sb = z2a_sb

                # transpose stats block: [125, tb] -> [tb, 125] (PSUM), to SBUF
                pst = pst_pool.tile([KR * BLK_T, P], F32)
                nc.tensor.transpose(
                    out=pst[:tb, :],
                    in_=statblk[:, b0 : b0 + T, :].rearrange("p t k -> p (t k)"),
                    identity=id_sb,
                )
                s2 = s2p.tile([KR * BLK_T, P], F32R)
                nc.scalar.copy(out=s2[:tb, :], in_=pst[:tb, :])

                # affine (mean + std*z) via full-rate f32r PE; two matmuls
                # (one PSUM bank each) share one 1024-wide sigmoid on ACT
                g0 = 0
                while g0 < T:
                    gw = min(2 * SIG_T, T - g0)          # 8 or tail 4 tiles
                    pa = paff_pool.tile([P, 2, SIG_T * NS], F32)
                    for k in range(gw // SIG_T):
                        nc.tensor.matmul(
                            pa[:, k, :],
                            lhsT=s2[:tb, :],
                            rhs=z2_sb[
                                :tb,
                                (g0 + k * SIG_T) * NS : (g0 + (k + 1) * SIG_T) * NS,
                            ],
                            start=True,
                            stop=True,
                        )
                    nc.scalar.activation(
                        out=outb[:, b0 + g0 : b0 + g0 + gw, :].rearrange(
                            "p t s -> p (t s)"
                        ),
                        in_=pa.rearrange("p a b -> p (a b)")[:, : gw * NS],
                        func=mybir.ActivationFunctionType.Sigmoid,
                    )
                    g0 += gw
            _split_dma(nc, oc, outb[:, :TC, :])

    nc.finalize()
    return nc


def _trunc_f32r(x: np.ndarray) -> np.ndarray:
    return (
        np.ascontiguousarray(x, dtype=np.float32).view(np.uint32) & np.uint32(F32R_MASK)
    ).view(np.float32)


def _z2_const(z: np.ndarray, T: int) -> np.ndarray:
    """Block-diagonal affine matrix, t-major rows (t*KR + k)."""
    zh = _trunc_f32r(z)
    zl = _trunc_f32r(z - zh)
    ones = np.ones(NS, dtype=np.float32)
    rows = (ones, ones, zh, zl, zh)
    z2 = np.zeros((KR * T, T * NS), dtype=np.float32)
    for t in range(T):
        c = slice(t * NS, (t + 1) * NS)
        for k in range(KR):
            z2[t * KR + k, c] = rows[k]
    return z2


def _host_consts(w_mu: np.ndarray, w_log_var: np.ndarray, z: np.ndarray):
    elv = np.exp(w_log_var.astype(np.float32))
    wmu_rep = np.tile(w_mu.astype(np.float32)[None, :], (P, 1))
    elv_rep = np.tile(elv[None, :], (P, 1))
    z = np.asarray(z, dtype=np.float32)
    return {
        "wmu": wmu_rep,
        "elv": elv_rep,
        "z2a": _z2_const(z, BLK_T),
        "ident": np.eye(P, dtype=np.float32),
    }


_PROGRAM_CACHE: dict[int, "bass.Bass"] = {}


def run(X, w_mu, w_log_var, z, trace=False):
    X = np.ascontiguousarray(X, dtype=np.float32)
    n = X.shape[0]
    assert n % N_CORES == 0
    rows = n // N_CORES
    if rows not in _PROGRAM_CACHE:
        _PROGRAM_CACHE[rows] = build_program(rows)
    nc = _PROGRAM_CACHE[rows]

    consts = _host_consts(np.asarray(w_mu), np.asarray(w_log_var), np.asarray(z))
    in_maps = [
        {"x": X[i * rows : (i + 1) * rows], **consts} for i in range(N_CORES)
    ]
    res = run_bass_kernel_spmd(nc, in_maps, list(range(N_CORES)), trace=trace)
    outs = [res.results[i]["out"] for i in range(N_CORES)]
    full = np.concatenate(outs, axis=0)
    return full, res


def kernel(X, w_mu, w_log_var, z):
    full, _ = run(X, w_mu, w_log_var, z, trace=False)
    return full


# revision 14
# speedup vs baseline: 1.0285x; 1.0285x over previous
"""Trainium2 Bass kernel: sampled logistic-regression forward.

reference math (per data row i, sample s):
    mean_i = X[i] . w_mu
    var_i  = sum_d X[i,d]^2 * exp(w_log_var[d])
    out[i,s] = sigmoid( sqrt(var_i) * z[s] + mean_i )

Full shapes: X [500000, 64], w_mu [64], w_log_var [64], z [128]
Output: [500000, 128] fp32.

Sharding: data-parallel over 8 NeuronCores, 62500 rows each.

Layout: chunk-local stripe. Chunk c covers shard rows
[c*125*SC, c*125*SC + 125*TC) -- one contiguous DRAM range per DMA --
and within the chunk, partition p holds rows chunk_base + p*TC + t.
Each DMA descriptor is a per-partition contiguous run of TC rows
(12-24 KB) and each DMA instruction's DRAM side is one contiguous
1.5-3 MB range. Streaming DMAs are additionally split into a
120-partition instruction + 5-partition instruction: descriptor counts
divisible by 15 fan out across all 15 DMA engines (125-descriptor
transfers were observed pinned to 5 engines at ~27 GB/s each).

Per-core pipeline, super-chunks of SC=48 tiles x [125 rows, 64]:
  - DMA in X chunk (contiguous, 120+5 split)
  - ACT: X2 = Square(X)            (sigmoid_and_others table set)
  - DVE: A = X * w_mu (materialized rep); reduce A -> mean;
    reduce V -> var
  - GPSIMD: V = X2 * exp(lv) (rep, in place); Newton rsqrt
    (bit-trick seed, 2 iters); std = var * y
  - stats split hi/lo to f32r precision (mantissa AND-mask) into a
    k-major stats block [125, 5, SC] (contiguous writes)
  - per 24-tile block: PE transpose stats (f32r identity, full rate)
    -> [120, 125]; full-rate float32r matmuls vs constant
    block-diagonal Z2 [5*T, T*128] with rows ordered k*T+t:
      arg = mh*1 + ml*1 + sh*zh + sh*zl + sl*zh  ~= mean + std*z (~2^-24)
  - ACT: Sigmoid [125, 1024] PSUM->SBUF (paired matmul banks)
  - DMA out chunk (contiguous, 120+5 split)
"""

from contextlib import ExitStack

import numpy as np

import concourse.bacc as bacc
import concourse.bass as bass
import concourse.tile as tile
from concourse import mybir
from concourse.bass_utils import run_bass_kernel_spmd

N_CORES = 8
D = 64
NS = 128
P = 125          # rows per tile (partition dim)
SC = 48          # tiles per super-chunk (DMA + stats granularity)
BLK_T = 24       # tiles per matmul block (5*24 = 120 = K of the affine matmul)
SIG_T = 4        # tiles per PSUM bank (4*128 = 512 f32)
KR = 5           # K-rows per tile: mh, ml, sh(*zh), sh(*zl), sl(*zh)
PSPLIT = 120     # descriptor-count split: 120 (15 engines) + 5

RSQRT_MAGIC = 0x5F3759DF
F32R_MASK = 0xFFFFF000   # keep 11 explicit mantissa bits (f32r-representable)
F32 = mybir.dt.float32
F32R = mybir.dt.float32r
BF16 = mybir.dt.bfloat16
U32 = mybir.dt.uint32


def _split_dma(nc, out_tile, in_ap, eng=None):
    eng = eng or nc.sync
    eng.dma_start(out=out_tile[0:PSPLIT], in_=in_ap[0:PSPLIT])
    eng.dma_start(out=out_tile[PSPLIT:P], in_=in_ap[PSPLIT:P])


def build_program(rows: int):
    """Build the single-core Bass/Tile program for `rows` rows (SPMD across cores)."""
    assert rows % P == 0
    ntiles = rows // P
    assert ntiles % SIG_T == 0
    RT = ntiles % BLK_T          # rump matmul-block size (0 -> none)

    nc = bacc.Bacc(
        "TRN2",
        target_bir_lowering=False,
        debug=False,
        num_devices=N_CORES,
    )

    x = nc.dram_tensor("x", [rows, D], F32, kind="ExternalInput")
    wmu_d = nc.dram_tensor("wmu", [P, SC * D], F32, kind="ExternalInput")
    elv_d = nc.dram_tensor("elv", [P, SC * D], F32, kind="ExternalInput")
    z2a_d = nc.dram_tensor("z2a", [KR * BLK_T, BLK_T * NS], F32R, kind="ExternalInput")
    ident = nc.dram_tensor("ident", [P, P], F32, kind="ExternalInput")
    out = nc.dram_tensor("out", [rows, NS], F32, kind="ExternalOutput")

    with tile.TileContext(nc) as tc, ExitStack() as ctx:
        singles = ctx.enter_context(tc.tile_pool(name="singles", bufs=1))
        xin = ctx.enter_context(tc.tile_pool(name="xin", bufs=4))
        sqp = ctx.enter_context(tc.tile_pool(name="sqp", bufs=2))
        amp = ctx.enter_context(tc.tile_pool(name="amp", bufs=2))
        statp = ctx.enter_context(tc.tile_pool(name="statp", bufs=4))
        smalls = ctx.enter_context(tc.tile_pool(name="smalls", bufs=5))
        s2p = ctx.enter_context(tc.tile_pool(name="s2p", bufs=4))
        outp = ctx.enter_context(tc.tile_pool(name="outp", bufs=2))
        pst_pool = ctx.enter_context(tc.tile_pool(name="pst", bufs=2, space="PSUM"))
        paff_pool = ctx.enter_context(tc.tile_pool(name="paff", bufs=3, space="PSUM"))

        # one-time loads; weight vectors are materialized as full [P, SC, D]
        # tensors so the big per-chunk muls avoid stride-0 broadcast APs
        wmu_rep = singles.tile([P, SC, D], F32)
        nc.sync.dma_start(out=wmu_rep, in_=wmu_d.rearrange("p (t d) -> p t d", d=D))
        elv_rep = singles.tile([P, SC, D], F32)
        nc.sync.dma_start(out=elv_rep, in_=elv_d.rearrange("p (t d) -> p t d", d=D))
        z2a_sb = singles.tile([KR * BLK_T, BLK_T * NS], F32R)
        nc.sync.dma_start(out=z2a_sb, in_=z2a_d[:, :])
        id_sb = singles.tile([P, P], F32)
        nc.sync.dma_start(out=id_sb, in_=ident[:, :])
        magic_sb = singles.tile([P, SC], U32)
        nc.vector.memset(magic_sb, RSQRT_MAGIC)
        one_sb = singles.tile([P, 1], U32)
        nc.vector.memset(one_sb, 1)
        mask_sb = singles.tile([P, 1], U32)
        nc.vector.memset(mask_sb, F32R_MASK)

        sched = []
        rem = ntiles
        for s_ in (12, 12, 24):
            if rem >= s_ + SC:
                sched.append(s_)
                rem -= s_
        while rem > SC:
            sched.append(SC)
            rem -= SC
        if rem:
            sched.append(rem)
        c0 = 0
        for TC in sched:
            row0 = c0 * P
            xc = x[row0 : row0 + P * TC, :].rearrange("(p t) d -> p t d", p=P)
            oc = out[row0 : row0 + P * TC, :].rearrange("(p t) s -> p t s", p=P)

            xt = xin.tile([P, SC, D], F32)
            _split_dma(nc, xt[:, :TC, :], xc, eng=nc.scalar)

            # X^2 on ACT (Square lives in the sigmoid table set)
            x2 = sqp.tile([P, SC, D], F32)
            nc.scalar.activation(
                out=x2[:, :TC, :], in_=xt[:, :TC, :],
                func=mybir.ActivationFunctionType.Square,
            )
            # A = X * w_mu on DVE
            at = amp.tile([P, SC, D], F32)
            nc.vector.tensor_mul(at[:, :TC, :], xt[:, :TC, :], wmu_rep[:, :TC, :])
            # V = X^2 * exp(lv) in place on GPSIMD
            nc.gpsimd.tensor_mul(
                x2[:, :TC, :], x2[:, :TC, :], elv_rep[:, :TC, :]
            )

            mean_t = smalls.tile([P, SC], F32)
            nc.vector.tensor_reduce(
                out=mean_t[:, :TC],
                in_=at[:, :TC, :],
                axis=mybir.AxisListType.X,
                op=mybir.AluOpType.add,
            )
            var = smalls.tile([P, SC], F32)
            nc.vector.tensor_reduce(
                out=var[:, :TC],
                in_=x2[:, :TC, :],
                axis=mybir.AxisListType.X,
                op=mybir.AluOpType.add,
            )

            # y = rsqrt(var) on GPSIMD: seed 0x5f3759df - (bits >> 1), 2 NR iters
            vb = var[:, :TC].bitcast(U32)
            yb = smalls.tile([P, SC], U32)
            nc.vector.tensor_scalar(
                yb[:, :TC], vb, one_sb[:, 0:1], None,
                op0=mybir.AluOpType.logical_shift_right,
            )
            nc.vector.scalar_tensor_tensor(
                out=yb[:, :TC],
                in0=magic_sb[:, :TC],
                scalar=0,
                in1=yb[:, :TC],
                op0=mybir.AluOpType.bypass,
                op1=mybir.AluOpType.subtract,
            )
            y = yb.bitcast(F32)
            t2 = smalls.tile([P, SC], F32)
            for _ in range(2):
                # y <- y*(1.5 - 0.5*var*y^2), via u=y*y; h=(u*-0.5)*var;
                # y=(h+1.5)*y
                nc.gpsimd.tensor_mul(t2[:, :TC], y[:, :TC], y[:, :TC])
                nc.vector.scalar_tensor_tensor(
                    out=t2[:, :TC], in0=t2[:, :TC], scalar=-0.5, in1=var[:, :TC],
                    op0=mybir.AluOpType.mult, op1=mybir.AluOpType.mult,
                )
                nc.vector.scalar_tensor_tensor(
                    out=y[:, :TC], in0=t2[:, :TC], scalar=1.5, in1=y[:, :TC],
                    op0=mybir.AluOpType.add, op1=mybir.AluOpType.mult,
                )
            std_t = smalls.tile([P, SC], F32)
            nc.gpsimd.tensor_mul(std_t[:, :TC], var[:, :TC], y[:, :TC])

            # split mean/std into f32r-representable hi/lo rows:
            # statblk rows per tile: [mh, ml, sh, sh, sl] (t-major, k fastest)
            statblk = statp.tile([P, SC, KR], F32)
            sb_u = statblk.bitcast(U32)
            rem = smalls.tile([P, SC], F32)
            rem2 = smalls.tile([P, SC], F32)
            nc.vector.tensor_scalar(
                sb_u[:, :TC, 0], mean_t[:, :TC].bitcast(U32), mask_sb[:, 0:1], None,
                op0=mybir.AluOpType.bitwise_and,
            )
            nc.vector.tensor_sub(rem[:, :TC], mean_t[:, :TC], statblk[:, :TC, 0])
            nc.vector.tensor_scalar(
                sb_u[:, :TC, 1], rem[:, :TC].bitcast(U32), mask_sb[:, 0:1], None,
                op0=mybir.AluOpType.bitwise_and,
            )
            nc.vector.tensor_scalar(
                sb_u[:, :TC, 2], std_t[:, :TC].bitcast(U32), mask_sb[:, 0:1], None,
                op0=mybir.AluOpType.bitwise_and,
            )
            # same-engine as the sh mask write: the strided k-slice
            # cross-engine dependency is not reliably enforced (observed
            # stale row-3 reads when these ran on gpsimd)
            nc.vector.tensor_copy(sb_u[:, :TC, 3], sb_u[:, :TC, 2])
            nc.vector.tensor_sub(rem2[:, :TC], std_t[:, :TC], statblk[:, :TC, 2])
            nc.vector.tensor_scalar(
                sb_u[:, :TC, 4], rem2[:, :TC].bitcast(U32), mask_sb[:, 0:1], None,
                op0=mybir.AluOpType.bitwise_and,
            )

            outb = outp.tile([P, SC, NS], F32)
            for b0 in range(0, TC, BLK_T):
                T = min(BLK_T, TC - b0)
                tb = KR * T
                z2_sb = z2a_sb

                # transpose stats block: [125, tb] -> [tb, 125] (PSUM), to SBUF
                pst = pst_pool.tile([KR * BLK_T, P], F32)
                nc.tensor.transpose(
                    out=pst[:tb, :],
                    in_=statblk[:, b0 : b0 + T, :].rearrange("p t k -> p (t k)"),
                    identity=id_sb,
                )
                s2 = s2p.tile([KR * BLK_T, P], F32R)
                nc.scalar.copy(out=s2[:tb, :], in_=pst[:tb, :])

                # affine (mean + std*z) via full-rate f32r PE; two matmuls
                # (one PSUM bank each) share one 1024-wide sigmoid on ACT
                g0 = 0
                while g0 < T:
                    gw = min(2 * SIG_T, T - g0)          # 8 or tail 4 tiles
                    pa = paff_pool.tile([P, 2, SIG_T * NS], F32)
                    for k in range(gw // SIG_T):
                        nc.tensor.matmul(
                            pa[:, k, :],
                            lhsT=s2[:tb, :],
                            rhs=z2_sb[
                                :tb,
                                (g0 + k * SIG_T) * NS : (g0 + (k + 1) * SIG_T) * NS,
                            ],
                            start=True,
                            stop=True,
                        )
                    nc.scalar.activation(
                        out=outb[:, b0 + g0 : b0 + g0 + gw, :].rearrange(
                            "p t s -> p (t s)"
                        ),
                        in_=pa.rearrange("p a b -> p (a b)")[:, : gw * NS],
                        func=mybir.ActivationFunctionType.Sigmoid,
                    )
                    g0 += gw
            _split_dma(nc, oc, outb[:, :TC, :])
            c0 += TC

    nc.finalize()
    return nc


def _trunc_f32r(x: np.ndarray) -> np.ndarray:
    return (
        np.ascontiguousarray(x, dtype=np.float32).view(np.uint32) & np.uint32(F32R_MASK)
    ).view(np.float32)


def _z2_const(z: np.ndarray, T: int) -> np.ndarray:
    """Block-diagonal affine matrix, t-major rows (t*KR + k)."""
    zh = _trunc_f32r(z)
    zl = _trunc_f32r(z - zh)
    ones = np.ones(NS, dtype=np.float32)
    rows = (ones, ones, zh, zl, zh)
    z2 = np.zeros((KR * T, T * NS), dtype=np.float32)
    for t in range(T):
        c = slice(t * NS, (t + 1) * NS)
        for k in range(KR):
            z2[t * KR + k, c] = rows[k]
    return z2


def _host_consts(w_mu: np.ndarray, w_log_var: np.ndarray, z: np.ndarray):
    elv = np.exp(w_log_var.astype(np.float32))
    wmu_rep = np.tile(w_mu.astype(np.float32)[None, :], (P, SC))
    elv_rep = np.tile(elv[None, :], (P, SC))
    z = np.asarray(z, dtype=np.float32)
    return {
        "wmu": wmu_rep,
        "elv": elv_rep,
        "z2a": _z2_const(z, BLK_T),
        "ident": np.eye(P, dtype=np.float32),
    }


_PROGRAM_CACHE: dict[int, "bass.Bass"] = {}


def run(X, w_mu, w_log_var, z, trace=False):
    X = np.ascontiguousarray(X, dtype=np.float32)
    n = X.shape[0]
    assert n % N_CORES == 0
    rows = n // N_CORES
    if rows not in _PROGRAM_CACHE:
        _PROGRAM_CACHE[rows] = build_program(rows)
    nc = _PROGRAM_CACHE[rows]

    consts = _host_consts(np.asarray(w_mu), np.asarray(w_log_var), np.asarray(z))
    in_maps = [
        {"x": X[i * rows : (i + 1) * rows], **consts} for i in range(N_CORES)
    ]
    res = run_bass_kernel_spmd(nc, in_maps, list(range(N_CORES)), trace=trace)
    outs = [res.results[i]["out"] for i in range(N_CORES)]
    full = np.concatenate(outs, axis=0)
    return full, res


def kernel(X, w_mu, w_log_var, z):
    full, _ = run(X, w_mu, w_log_var, z, trace=False)
    return full


# revision 15
# speedup vs baseline: 1.0341x; 1.0054x over previous
"""Trainium2 Bass kernel: sampled logistic-regression forward.

reference math (per data row i, sample s):
    mean_i = X[i] . w_mu
    var_i  = sum_d X[i,d]^2 * exp(w_log_var[d])
    out[i,s] = sigmoid( sqrt(var_i) * z[s] + mean_i )

Full shapes: X [500000, 64], w_mu [64], w_log_var [64], z [128]
Output: [500000, 128] fp32.

Sharding: data-parallel over 8 NeuronCores, 62500 rows each.

Layout: chunk-local stripe. Chunk c covers shard rows
[c*125*SC, c*125*SC + 125*TC) -- one contiguous DRAM range per DMA --
and within the chunk, partition p holds rows chunk_base + p*TC + t.
Each DMA descriptor is a per-partition contiguous run of TC rows
(12-24 KB) and each DMA instruction's DRAM side is one contiguous
1.5-3 MB range. Streaming DMAs are additionally split into a
120-partition instruction + 5-partition instruction: descriptor counts
divisible by 15 fan out across all 15 DMA engines (125-descriptor
transfers were observed pinned to 5 engines at ~27 GB/s each).

Input DMAs issue on the scalar (ACT) HWDGE queue and output DMAs on the
sync queue so the two transfer chains overlap. NOTE: DMA engines E69+
round f32 payloads to f32r (11-bit mantissa) on the sync queue's ring;
X therefore loads via the scalar queue (observed exact), and the f32r
rounding of the sigmoid outputs on the out path is harmless (<2.4e-4).
The chunk schedule is graded (12, 12, 24, 48...) so the first out-DMA
fires early -- pipeline fill was ~75us with uniform 48-tile chunks.

Per-core pipeline, super-chunks of SC=48 tiles x [125 rows, 64]:
  - DMA in X chunk (contiguous, 120+5 split)
  - ACT: X2 = Square(X)            (sigmoid_and_others table set)
  - DVE: A = X * w_mu (materialized rep); reduce A -> mean;
    reduce V -> var
  - GPSIMD: V = X2 * exp(lv) (rep, in place); Newton rsqrt
    (bit-trick seed, 2 iters); std = var * y
  - stats split hi/lo to f32r precision (mantissa AND-mask) into a
    k-major stats block [125, 5, SC] (contiguous writes)
  - per 24-tile block: PE transpose stats (f32r identity, full rate)
    -> [120, 125]; full-rate float32r matmuls vs constant
    block-diagonal Z2 [5*T, T*128] with rows ordered k*T+t:
      arg = mh*1 + ml*1 + sh*zh + sh*zl + sl*zh  ~= mean + std*z (~2^-24)
  - ACT: Sigmoid [125, 1024] PSUM->SBUF (paired matmul banks)
  - DMA out chunk (contiguous, 120+5 split)
"""

from contextlib import ExitStack

import numpy as np

import concourse.bacc as bacc
import concourse.bass as bass
import concourse.tile as tile
from concourse import mybir
from concourse.bass_utils import run_bass_kernel_spmd

N_CORES = 8
D = 64
NS = 128
P = 125          # rows per tile (partition dim)
SC = 48          # tiles per super-chunk (DMA + stats granularity)
BLK_T = 24       # tiles per matmul block (5*24 = 120 = K of the affine matmul)
SIG_T = 4        # tiles per PSUM bank (4*128 = 512 f32)
KR = 5           # K-rows per tile: mh, ml, sh(*zh), sh(*zl), sl(*zh)
PSPLIT = 120     # descriptor-count split: 120 (15 engines) + 5

RSQRT_MAGIC = 0x5F3759DF
F32R_MASK = 0xFFFFF000   # keep 11 explicit mantissa bits (f32r-representable)
F32 = mybir.dt.float32
F32R = mybir.dt.float32r
BF16 = mybir.dt.bfloat16
U32 = mybir.dt.uint32


def _split_dma(nc, out_tile, in_ap, eng=None):
    eng = eng or nc.sync
    eng.dma_start(out=out_tile[0:PSPLIT], in_=in_ap[0:PSPLIT])
    eng.dma_start(out=out_tile[PSPLIT:P], in_=in_ap[PSPLIT:P])


def build_program(rows: int):
    """Build the single-core Bass/Tile program for `rows` rows (SPMD across cores)."""
    assert rows % P == 0
    ntiles = rows // P
    assert ntiles % SIG_T == 0
    RT = ntiles % BLK_T          # rump matmul-block size (0 -> none)

    nc = bacc.Bacc(
        "TRN2",
        target_bir_lowering=False,
        debug=False,
        num_devices=N_CORES,
    )

    x = nc.dram_tensor("x", [rows, D], F32, kind="ExternalInput")
    wmu_d = nc.dram_tensor("wmu", [P, SC * D], F32, kind="ExternalInput")
    elv_d = nc.dram_tensor("elv", [P, SC * D], F32, kind="ExternalInput")
    z2a_d = nc.dram_tensor("z2a", [KR * BLK_T, BLK_T * NS], F32R, kind="ExternalInput")
    ident = nc.dram_tensor("ident", [P, P], F32, kind="ExternalInput")
    out = nc.dram_tensor("out", [rows, NS], F32, kind="ExternalOutput")

    with tile.TileContext(nc) as tc, ExitStack() as ctx:
        singles = ctx.enter_context(tc.tile_pool(name="singles", bufs=1))
        xin = ctx.enter_context(tc.tile_pool(name="xin", bufs=4))
        sqp = ctx.enter_context(tc.tile_pool(name="sqp", bufs=2))
        amp = ctx.enter_context(tc.tile_pool(name="amp", bufs=2))
        statp = ctx.enter_context(tc.tile_pool(name="statp", bufs=4))
        smalls = ctx.enter_context(tc.tile_pool(name="smalls", bufs=5))
        s2p = ctx.enter_context(tc.tile_pool(name="s2p", bufs=4))
        outp = ctx.enter_context(tc.tile_pool(name="outp", bufs=2))
        pst_pool = ctx.enter_context(tc.tile_pool(name="pst", bufs=2, space="PSUM"))
        paff_pool = ctx.enter_context(tc.tile_pool(name="paff", bufs=3, space="PSUM"))

        # one-time loads; weight vectors are materialized as full [P, SC, D]
        # tensors so the big per-chunk muls avoid stride-0 broadcast APs
        wmu_rep = singles.tile([P, SC, D], F32)
        nc.sync.dma_start(out=wmu_rep, in_=wmu_d.rearrange("p (t d) -> p t d", d=D))
        elv_rep = singles.tile([P, SC, D], F32)
        nc.sync.dma_start(out=elv_rep, in_=elv_d.rearrange("p (t d) -> p t d", d=D))
        z2a_sb = singles.tile([KR * BLK_T, BLK_T * NS], F32R)
        nc.sync.dma_start(out=z2a_sb, in_=z2a_d[:, :])
        id_sb = singles.tile([P, P], F32)
        nc.sync.dma_start(out=id_sb, in_=ident[:, :])
        magic_sb = singles.tile([P, SC], U32)
        nc.vector.memset(magic_sb, RSQRT_MAGIC)
        one_sb = singles.tile([P, 1], U32)
        nc.vector.memset(one_sb, 1)
        mask_sb = singles.tile([P, 1], U32)
        nc.vector.memset(mask_sb, F32R_MASK)

        sched = []
        rem = ntiles
        for s_ in (12, 12, 24):
            if rem >= s_ + SC:
                sched.append(s_)
                rem -= s_
        while rem > SC:
            sched.append(SC)
            rem -= SC
        if rem:
            sched.append(rem)
        c0 = 0
        for TC in sched:
            row0 = c0 * P
            xc = x[row0 : row0 + P * TC, :].rearrange("(p t) d -> p t d", p=P)
            oc = out[row0 : row0 + P * TC, :].rearrange("(p t) s -> p t s", p=P)

            xt = xin.tile([P, SC, D], F32)
            _split_dma(nc, xt[:, :TC, :], xc, eng=nc.scalar)

            # X^2 on ACT (Square lives in the sigmoid table set)
            x2 = sqp.tile([P, SC, D], F32)
            nc.scalar.activation(
                out=x2[:, :TC, :], in_=xt[:, :TC, :],
                func=mybir.ActivationFunctionType.Square,
            )
            # A = X * w_mu on DVE
            at = amp.tile([P, SC, D], F32)
            nc.vector.tensor_mul(at[:, :TC, :], xt[:, :TC, :], wmu_rep[:, :TC, :])
            # V = X^2 * exp(lv) in place on GPSIMD
            nc.gpsimd.tensor_mul(
                x2[:, :TC, :], x2[:, :TC, :], elv_rep[:, :TC, :]
            )

            mean_t = smalls.tile([P, SC], F32)
            nc.vector.tensor_reduce(
                out=mean_t[:, :TC],
                in_=at[:, :TC, :],
                axis=mybir.AxisListType.X,
                op=mybir.AluOpType.add,
            )
            var = smalls.tile([P, SC], F32)
            nc.vector.tensor_reduce(
                out=var[:, :TC],
                in_=x2[:, :TC, :],
                axis=mybir.AxisListType.X,
                op=mybir.AluOpType.add,
            )

            # y = rsqrt(var) on GPSIMD: seed 0x5f3759df - (bits >> 1), 2 NR iters
            vb = var[:, :TC].bitcast(U32)
            yb = smalls.tile([P, SC], U32)
            nc.vector.tensor_scalar(
                yb[:, :TC], vb, one_sb[:, 0:1], None,
                op0=mybir.AluOpType.logical_shift_right,
            )
            nc.vector.scalar_tensor_tensor(
                out=yb[:, :TC],
                in0=magic_sb[:, :TC],
                scalar=0,
                in1=yb[:, :TC],
                op0=mybir.AluOpType.bypass,
                op1=mybir.AluOpType.subtract,
            )
            y = yb.bitcast(F32)
            t2 = smalls.tile([P, SC], F32)
            for _ in range(2):
                # y <- y*(1.5 - 0.5*var*y^2), via u=y*y; h=(u*-0.5)*var;
                # y=(h+1.5)*y
                nc.gpsimd.tensor_mul(t2[:, :TC], y[:, :TC], y[:, :TC])
                nc.vector.scalar_tensor_tensor(
                    out=t2[:, :TC], in0=t2[:, :TC], scalar=-0.5, in1=var[:, :TC],
                    op0=mybir.AluOpType.mult, op1=mybir.AluOpType.mult,
                )
                nc.vector.scalar_tensor_tensor(
                    out=y[:, :TC], in0=t2[:, :TC], scalar=1.5, in1=y[:, :TC],
                    op0=mybir.AluOpType.add, op1=mybir.AluOpType.mult,
                )
            std_t = smalls.tile([P, SC], F32)
            nc.gpsimd.tensor_mul(std_t[:, :TC], var[:, :TC], y[:, :TC])

            # split mean/std into f32r-representable hi/lo rows:
            # statblk rows per tile: [mh, ml, sh, sh, sl] (t-major, k fastest)
            statblk = statp.tile([P, SC, KR], F32)
            sb_u = statblk.bitcast(U32)
            rem = smalls.tile([P, SC], F32)
            rem2 = smalls.tile([P, SC], F32)
            nc.vector.tensor_scalar(
                sb_u[:, :TC, 0], mean_t[:, :TC].bitcast(U32), mask_sb[:, 0:1], None,
                op0=mybir.AluOpType.bitwise_and,
            )
            nc.vector.tensor_sub(rem[:, :TC], mean_t[:, :TC], statblk[:, :TC, 0])
            nc.vector.tensor_scalar(
                sb_u[:, :TC, 1], rem[:, :TC].bitcast(U32), mask_sb[:, 0:1], None,
                op0=mybir.AluOpType.bitwise_and,
            )
            nc.vector.tensor_scalar(
                sb_u[:, :TC, 2], std_t[:, :TC].bitcast(U32), mask_sb[:, 0:1], None,
                op0=mybir.AluOpType.bitwise_and,
            )
            # same-engine as the sh mask write: the strided k-slice
            # cross-engine dependency is not reliably enforced (observed
            # stale row-3 reads when these ran on gpsimd)
            nc.vector.tensor_copy(sb_u[:, :TC, 3], sb_u[:, :TC, 2])
            nc.vector.tensor_sub(rem2[:, :TC], std_t[:, :TC], statblk[:, :TC, 2])
            nc.vector.tensor_scalar(
                sb_u[:, :TC, 4], rem2[:, :TC].bitcast(U32), mask_sb[:, 0:1], None,
                op0=mybir.AluOpType.bitwise_and,
            )

            outb = outp.tile([P, SC, NS], F32)
            for b0 in range(0, TC, BLK_T):
                T = min(BLK_T, TC - b0)
                tb = KR * T
                z2_sb = z2a_sb

                # transpose stats block: [125, tb] -> [tb, 125] (PSUM), to SBUF
                pst = pst_pool.tile([KR * BLK_T, P], F32)
                nc.tensor.transpose(
                    out=pst[:tb, :],
                    in_=statblk[:, b0 : b0 + T, :].rearrange("p t k -> p (t k)"),
                    identity=id_sb,
                )
                s2 = s2p.tile([KR * BLK_T, P], F32R)
                nc.scalar.copy(out=s2[:tb, :], in_=pst[:tb, :])

                # affine (mean + std*z) via full-rate f32r PE; two matmuls
                # (one PSUM bank each) share one 1024-wide sigmoid on ACT
                g0 = 0
                while g0 < T:
                    gw = min(2 * SIG_T, T - g0)          # 8 or tail 4 tiles
                    pa = paff_pool.tile([P, 2, SIG_T * NS], F32)
                    for k in range(gw // SIG_T):
                        nc.tensor.matmul(
                            pa[:, k, :],
                            lhsT=s2[:tb, :],
                            rhs=z2_sb[
                                :tb,
                                (g0 + k * SIG_T) * NS : (g0 + (k + 1) * SIG_T) * NS,
                            ],
                            start=True,
                            stop=True,
                        )
                    nc.scalar.activation(
                        out=outb[:, b0 + g0 : b0 + g0 + gw, :].rearrange(
                            "p t s -> p (t s)"
                        ),
                        in_=pa.rearrange("p a b -> p (a b)")[:, : gw * NS],
                        func=mybir.ActivationFunctionType.Sigmoid,
                    )
                    g0 += gw
            _split_dma(nc, oc, outb[:, :TC, :])
            c0 += TC

    nc.finalize()
    return nc


def _trunc_f32r(x: np.ndarray) -> np.ndarray:
    return (
        np.ascontiguousarray(x, dtype=np.float32).view(np.uint32) & np.uint32(F32R_MASK)
    ).view(np.float32)


def _z2_const(z: np.ndarray, T: int) -> np.ndarray:
    """Block-diagonal affine matrix, t-major rows (t*KR + k)."""
    zh = _trunc_f32r(z)
    zl = _trunc_f32r(z - zh)
    ones = np.ones(NS, dtype=np.float32)
    rows = (ones, ones, zh, zl, zh)
    z2 = np.zeros((KR * T, T * NS), dtype=np.float32)
    for t in range(T):
        c = slice(t * NS, (t + 1) * NS)
        for k in range(KR):
            z2[t * KR + k, c] = rows[k]
    return z2


def _host_consts(w_mu: np.ndarray, w_log_var: np.ndarray, z: np.ndarray):
    elv = np.exp(w_log_var.astype(np.float32))
    wmu_rep = np.tile(w_mu.astype(np.float32)[None, :], (P, SC))
    elv_rep = np.tile(elv[None, :], (P, SC))
    z = np.asarray(z, dtype=np.float32)
    return {
        "wmu": wmu_rep,
        "elv": elv_rep,
        "z2a": _z2_const(z, BLK_T),
        "ident": np.eye(P, dtype=np.float32),
    }


_PROGRAM_CACHE: dict[int, "bass.Bass"] = {}


def run(X, w_mu, w_log_var, z, trace=False):
    X = np.ascontiguousarray(X, dtype=np.float32)
    n = X.shape[0]
    assert n % N_CORES == 0
    rows = n // N_CORES
    if rows not in _PROGRAM_CACHE:
        _PROGRAM_CACHE[rows] = build_program(rows)
    nc = _PROGRAM_CACHE[rows]

    consts = _host_consts(np.asarray(w_mu), np.asarray(w_log_var), np.asarray(z))
    in_maps = [
        {"x": X[i * rows : (i + 1) * rows], **consts} for i in range(N_CORES)
    ]
    res = run_bass_kernel_spmd(nc, in_maps, list(range(N_CORES)), trace=trace)
    outs = [res.results[i]["out"] for i in range(N_CORES)]
    full = np.concatenate(outs, axis=0)
    return full, res


def kernel(X, w_mu, w_log_var, z):
    full, _ = run(X, w_mu, w_log_var, z, trace=False)
    return full
